# revision 6
# baseline (speedup 1.0000x reference)
"""Trainium2 Bass kernel for nn_Net_89094801588965 (moe_routing), fp8 rev.

Data-parallel over batch on 8 NeuronCores; per-core layout puts features on
SBUF partitions and batch on the free dim (same scaffolding as the bf16
baseline: host-side task grouping/permutation, per-task column segments,
carpooled h2/h3 merge, warm-up bridge).

The shared MLP (fused [priv|fc1] L1, L2, L3) now runs in fp8(e4m3) with
DoubleRow perf mode: one PE instruction contracts TWO 128-row planes at 0.5
cycles/column (4x the per-plane bf16 rate).  Plain fp8 fails the 2e-2 error
budget (~5%/layer), so every operand is split hi/lo: a ~= ah + al with both
parts e4m3 in a shared pow2 scale; the product uses three terms
  a@W ~= ah@Wh + al@Wh + ah@Wl
measured end-to-end error 4.7e-3 (budget 2e-2).  Per m-tile the terms pack
into DR plane pairs:
  A-instrs j: planes (ah_j, al_j) x (Wh_j, Wh_j)      [terms 1+2]
  B-instrs q: planes (ah_2q, ah_2q+1) x (Wl_2q, Wl_2q+1)   [term 3]
K-tails (784 = 6*128+16, 400 = 3*128+16) pack all three 16-row fragments
into ONE plane at partition offsets 0/32/64 against a matching stationary
tail plane (zeros elsewhere), so no half-empty tail instructions.

PE cost per batch column: L1 5*6=30, L2 2.5*4=10, L3 2.5*2=5 cycles plus
bf16 heads h1 4 + merge 2 = 51 (vs 72 all-bf16).

Scales (all pow2, folded into ACT scale args / host-prescaled stationaries):
x and W scaled so absmax ~224; activation scales sampled on host with 4x
margin.  p and L3-out stay bf16 (head inputs) stored at psum scale with the
descale folded into wh1 on the host.  Residuals live at the parent scale --
their sub-normal floor (~1e-3 absolute at scale ~50) is negligible.

On-device split per fp8-consumed activation tile (t = relu(psum*alpha + b)
in bf16 on ACT; hh = fp8(t); hl = fp8(t - hh) via scalar_tensor_tensor on
DVE).  Tail planes are memset on GpSimd first (garbage rows would NaN the
matmul even against zero stationary rows).
"""

import sys

sys.path.insert(0, "/opt/trn_rl_repo")

import numpy as np
import ml_dtypes

import concourse.bass as bass
import concourse.mybir as mybir
import concourse.tile as tile
from concourse import bacc
from concourse.bass_utils import run_bass_kernel_spmd

F32 = mybir.dt.float32
BF16 = mybir.dt.bfloat16
FP8 = mybir.dt.float8e4
RELU = mybir.ActivationFunctionType.Relu
COPY = mybir.ActivationFunctionType.Copy
ALU = mybir.AluOpType
DR = mybir.MatmulPerfMode.DoubleRow
BF16NP = ml_dtypes.bfloat16
E4NP = ml_dtypes.float8_e4m3

B = 65536
D = 784
HID = 400
LAT = 256
T = 10
NCLS = 10
HW1 = 28                 # head hidden width
NCORES = 8
CH = 512                 # batch columns per full chunk

M1 = LAT + HID           # 656 fused L1 output (private | fc1)

_cache = {}


def _ceil_tiles(n):
    full, rem = divmod(n, 128)
    return [128] * full + ([rem] if rem else [])


L1_M = _ceil_tiles(M1)           # [128]*5 + [16]
L2_M = _ceil_tiles(HID)          # [128]*3 + [16]
L3_M = _ceil_tiles(LAT)          # [128, 128]
H1_K = 4                         # 2*LAT = 4 full k-tiles

# bias column layout inside the single [128, 42] bias tensor
BC_L1, BC_L2, BC_L3 = 0, 6, 10
BC_H1B, BC_H2B, BC_H3B = 12, 22, 32
NBC = 42

# fp8 plane counts: (full k planes, has_tail)
L1_NA, L1_NB = 7, 3              # 784 = 6 full + tail plane; B pairs of ah
L2_NA, L2_NB = 3, 2              # 400 = 3 full + tail (packed into B q=1)
XROWS = 14 * 128                 # x dram rows: (i=2) x (j=7) x 128


def _chunks(rp):
    """Chunk widths: full 512s, remainder tapered [rem+320, 128, 64] at the
    end -- the head-pipeline drain is a serial act+matmul chain on the final
    chunks, so its latency scales with their width.  fp8-DR runs at full
    rate at any width; only the 512 moving-dim cap matters."""
    full, rem = divmod(rp, CH)
    last = CH + rem
    return [CH] * (full - 1) + [last - 192, 128, 64]


def _segments(cs, cw, bounds):
    segs = []
    for t in range(T):
        lo, hi = max(cs, bounds[t]), min(cs + cw, bounds[t + 1])
        if hi > lo:
            segs.append((lo - cs, hi - cs, t))
    return segs


def _mk_schedule(widths, bounds):
    starts = [0]
    for cw in widths:
        starts.append(starts[-1] + cw)
    segs_all = []
    for ci, cw in enumerate(widths):
        s = _segments(starts[ci], cw, bounds)
        if ci == len(widths) - 1:
            fine = []
            for s0, s1, t in s:
                mid = s0 + 8 * ((s1 - s0) // 16)
                if s0 < mid < s1:
                    fine += [(s0, mid, t), (mid, s1, t)]
                else:
                    fine.append((s0, s1, t))
            s = fine
        segs_all.append(s)
    segs_all += [[], []]
    pair_cols = {}
    for ci in range(2, len(widths)):
        for sa, sb in zip(segs_all[ci - 1], segs_all[ci - 2]):
            pair_cols.setdefault((sa[2], sb[2]), len(pair_cols))
    return segs_all, pair_cols


def _build_program(widths, bounds):
    rp = sum(widths)
    segs_all, pair_cols = _mk_schedule(widths, bounds)
    np_ = max(1, len(pair_cols))
    nc = bacc.Bacc("TRN2", target_bir_lowering=False, debug=False,
                   num_devices=NCORES)

    xT_d = nc.dram_tensor("xT", [XROWS, rp], FP8, kind="ExternalInput")
    sa1_d = nc.dram_tensor("sa1", [2 * L1_NA * 128, M1], FP8,
                           kind="ExternalInput")
    sb1_d = nc.dram_tensor("sb1", [2 * L1_NB * 128, M1], FP8,
                           kind="ExternalInput")
    sa2_d = nc.dram_tensor("sa2", [2 * L2_NA * 128, HID], FP8,
                           kind="ExternalInput")
    sb2_d = nc.dram_tensor("sb2", [2 * L2_NB * 128, HID], FP8,
                           kind="ExternalInput")
    sa3_d = nc.dram_tensor("sa3", [2 * L2_NA * 128, LAT], FP8,
                           kind="ExternalInput")
    sb3_d = nc.dram_tensor("sb3", [2 * L2_NB * 128, LAT], FP8,
                           kind="ExternalInput")
    sc1_d = nc.dram_tensor("sc1", [2 * 3 * 128, 16], FP8,
                           kind="ExternalInput")
    sc2_d = nc.dram_tensor("sc2", [2 * 2 * 128, 16], FP8,
                           kind="ExternalInput")
    wh1_d = nc.dram_tensor("wh1", [2 * LAT, T * HW1], BF16,
                           kind="ExternalInput")
    wh2_d = nc.dram_tensor("wh2", [HW1, T * HW1], BF16, kind="ExternalInput")
    wh3_d = nc.dram_tensor("wh3", [64, T * NCLS], BF16, kind="ExternalInput")
    wh23_d = nc.dram_tensor("wh23", [64, 42 * np_], BF16,
                            kind="ExternalInput")
    bias_d = nc.dram_tensor("bias", [128, NBC], F32, kind="ExternalInput")
    scal_d = nc.dram_tensor("scal", [128, 2], F32, kind="ExternalInput")
    out_d = nc.dram_tensor("out", [NCLS, rp], F32, kind="ExternalOutput")

    starts = [0]
    for cw in widths:
        starts.append(starts[-1] + cw)

    with tile.TileContext(nc) as tc:
        with (
            tc.tile_pool(name="wp", bufs=1) as wp,
            tc.tile_pool(name="xp", bufs=3) as xp,
            tc.tile_pool(name="ap", bufs=4) as ap,
            tc.tile_pool(name="app", bufs=5) as app,
            tc.tile_pool(name="op", bufs=3) as op,
            tc.tile_pool(name="ps", bufs=8, space="PSUM") as ps,
        ):
            def load_w3d(dram, ksizes, ncols, tag, dt, pool=wp, col0=0):
                nk, kt = len(ksizes), ksizes[-1]
                t = pool.tile([128, nk, ncols], dt, tag=tag)
                nfull = nk - (1 if kt < 128 else 0)
                src_ = dram[0:128 * nfull, col0:col0 + ncols].rearrange(
                    "(j p) m -> p j m", p=128)
                nc.sync.dma_start(t[:, 0:nfull, :], src_)
                if kt < 128:
                    nc.sync.dma_start(
                        t[0:kt, nk - 1, :],
                        dram[128 * nfull:128 * nfull + kt,
                             col0:col0 + ncols])
                return t

            def load_planes(dram, nk, m, tag):
                t = wp.tile([128, 2, nk, m], FP8, tag=tag)
                nc.sync.dma_start(
                    t[:], dram[:].rearrange("(i j p) m -> p i j m",
                                            p=128, i=2))
                return t

            def load_x_chunk(cs, cw):
                t = xp.tile([128, 2, 7, cw], FP8, tag="x")
                nc.sync.dma_start(
                    t[:], xT_d[:, cs:cs + cw].rearrange(
                        "(i j p) m -> p i j m", p=128, i=2))
                return t

            # PE p-state warm-up bridge (see baseline): dummy matmuls span
            # the initial weight-DMA wait so the ramp is spent on junk.
            warm = wp.tile([128, 128], BF16, tag="warm")
            nc.vector.memset(warm[:], 0.0)
            wps = ps.tile([16, 128], F32, tag="ps", name="wps")
            for i in range(55):
                w_ = 16 if i < 14 else 128
                nc.tensor.matmul(wps[:, 0:w_], warm[:, 0:16], warm[:, 0:w_],
                                 start=True, stop=True)

            # Prologue on TWO hwdge queues: x0/bias/x1 on SP, all weights on
            # the (idle) ACT queue, plane-interleaved so chunk-0's j-th DR
            # matmuls start as soon as both j-streams land.
            sa1 = wp.tile([128, 2, L1_NA, M1], FP8, tag="sa1")
            x0 = xp.tile([128, 2, 7, widths[0]], FP8, tag="x")
            bias = None
            for j in range(7):
                for i in range(2):
                    r0 = (i * 7 + j) * 128
                    nc.sync.dma_start(
                        x0[:, i, j, :], xT_d[r0:r0 + 128, 0:widths[0]])
                    nc.scalar.dma_start(
                        sa1[:, i, j, :], sa1_d[r0:r0 + 128, :])
                if j == 0:
                    bias = wp.tile([128, NBC], F32, tag="bias")
                    nc.sync.dma_start(bias[:], bias_d[:])
                    scal = wp.tile([128, 2], F32, tag="scal")
                    nc.sync.dma_start(scal[:], scal_d[:])
            # x1 heads the third (GpSimd/SWDGE) queue so it lands while the
            # SP queue is still streaming x0 -- round 1 starts ~4us earlier.
            x1 = xp.tile([128, 2, 7, widths[1]], FP8, tag="x")
            c1 = slice(starts[1], starts[1] + widths[1])
            nc.gpsimd.dma_start(
                x1[:], xT_d[:, c1].rearrange("(i j p) m -> p i j m",
                                             p=128, i=2))

            # remaining weights on a THIRD queue (GpSimd/SWDGE) so SA1 owns
            # the ACT hwdge queue and finishes ~2x sooner during the fill.
            def load_planes_gp(dram, nk, m, tag):
                t = wp.tile([128, 2, nk, m], FP8, tag=tag)
                nc.gpsimd.dma_start(
                    t[:], dram[:].rearrange("(i j p) m -> p i j m",
                                            p=128, i=2))
                return t

            sc1 = load_planes_gp(sc1_d, 3, 16, "sc1")
            sc2 = load_planes_gp(sc2_d, 2, 16, "sc2")
            sb1 = load_planes_gp(sb1_d, L1_NB, M1, "sb1")
            sa2 = load_planes_gp(sa2_d, L2_NA, HID, "sa2")
            sb2 = load_planes_gp(sb2_d, L2_NB, HID, "sb2")
            sa3 = load_planes_gp(sa3_d, L2_NA, LAT, "sa3")
            sb3 = load_planes_gp(sb3_d, L2_NB, LAT, "sb3")
            wh1 = wp.tile([128, 4, T * HW1], BF16, tag="wh1")
            nc.gpsimd.dma_start(
                wh1[:], wh1_d[:].rearrange("(j p) m -> p j m", p=128))
            wh2 = wp.tile([HW1, T * HW1], BF16, tag="wh2")
            nc.gpsimd.dma_start(wh2[:], wh2_d[:])
            wh3 = wp.tile([64, T * NCLS], BF16, tag="wh3")
            nc.gpsimd.dma_start(wh3[:], wh3_d[:])
            wh23 = wp.tile([64, 42 * np_], BF16, tag="wh23")
            nc.gpsimd.dma_start(wh23[:], wh23_d[:])

            # ---- fp8 3-term DR matmul ----------------------------------
            def mm_dr(xk, sa, sb, msizes, cw, k_outer=False, sc=None):
                """A-instrs: planes (ah_j, al_j) x (Wh_j, Wh_j) for j in nA;
                B-instrs: planes (ah_2q, ah_2q+1) x Wl pairs.  Tail terms
                ride inside packed planes (see module docstring).  When sc is
                given, the LAST (16-wide) m-tile is computed plain-fp8 via
                C-instrs: ah plane-pairs x Wh pairs only -- 2x fewer
                instructions for 16/656 of the outputs (for L1 the packed
                k-tail instr is kept, its residual frags are host-free)."""
                nA, nB = sa.shape[2], sb.shape[2]
                offs = []
                c0 = 0
                for mp_ in msizes:
                    offs.append(c0)
                    c0 += mp_
                psums = [ps.tile([mp_, cw], F32, tag="ps", name="psm")
                         for mp_ in msizes]
                nm = len(msizes)

                def descs(mi):
                    if sc is not None and mi == nm - 1:
                        d = [('C', q) for q in range(sc.shape[2])]
                        if nA == 7:        # L1: packed-k-tail A-instr
                            d.append(('A', nA - 1))
                        return d
                    return ([('A', j) for j in range(nA)]
                            + [('B', q) for q in range(nB)])

                mains = [mi for mi in range(nm)
                         if not (sc is not None and mi == nm - 1)]
                if k_outer:
                    order = ([('A', j, mi) for j in range(nA)
                              for mi in mains]
                             + [('B', q, mi) for q in range(nB)
                                for mi in mains])
                    if sc is not None:
                        order += [(kind, k, nm - 1)
                                  for kind, k in descs(nm - 1)]
                else:
                    order = [(kind, k, mi) for mi in range(nm)
                             for kind, k in descs(mi)]
                seen = [0] * nm
                totals = [len(descs(mi)) for mi in range(nm)]
                for kind, k, mi in order:
                    mo, mp_ = offs[mi], msizes[mi]
                    if kind == 'A':
                        rhs = xk[:, :, k, :]
                        lhsT = sa[:, :, k, mo:mo + mp_]
                    elif kind == 'B':
                        rhs = xk[:, 0, 2 * k:2 * k + 2, :]
                        lhsT = sb[:, :, k, mo:mo + mp_]
                    else:
                        rhs = xk[:, 0, 2 * k:2 * k + 2, :]
                        lhsT = sc[:, :, k, 0:mp_]
                    seen[mi] += 1
                    nc.tensor.matmul(psums[mi][:], lhsT, rhs,
                                     start=(seen[mi] == 1),
                                     stop=(seen[mi] == totals[mi]),
                                     perf_mode=DR)
                return psums

            # ---- activation stages -------------------------------------
            def act_bf16(psums, bcol, msizes, tag, cw, eng="ad", pool=None):
                """Baseline-style relu+bias to bf16 (psum-scale folded into
                downstream stationaries on host)."""
                outs = []
                pool = pool or ap
                for mi, mp_ in enumerate(msizes):
                    t = pool.tile([mp_, cw], BF16, tag=f"{tag}{mi}")
                    bap = bias[:mp_, bcol + mi:bcol + mi + 1]
                    if eng[mi % len(eng)] == "a":
                        nc.scalar.activation(t[:], psums[mi][:], RELU,
                                             bias=bap, scale=1.0)
                    else:
                        nc.vector.tensor_scalar(
                            t[:], psums[mi][:], bap, 0.0,
                            op0=ALU.add, op1=ALU.max)
                    outs.append(t)
                return outs

            def act_split(psums, bcol, scol, tag, cw, hh_on_act=False):
                """psums: [128]*3 + [16]; produce fp8 split tile
                [128, 2, 4, cw]: (0,j)=hh_j, (1,j)=hl_j for full planes; the
                16-row k-tail at (0,3)[0:16] is plain fp8 (no residual --
                direct ACT write from psum; rest of the plane memset so the
                matmul never reads garbage)."""
                h8 = ap.tile([128, 2, 4, cw], FP8, tag=f"{tag}8")
                nc.gpsimd.memset(h8[:, 0, 3, :], 0.0)
                tb = ap.tile([128, 3, cw], BF16, tag=f"{tag}t")
                for j in range(3):
                    nc.scalar.activation(
                        tb[:, j, :], psums[j][:], RELU,
                        bias=bias[:128, bcol + j:bcol + j + 1],
                        scale=scal[0:128, scol:scol + 1])
                if hh_on_act:
                    nc.scalar.activation(h8[:, 0, 0:3, :], tb[:], COPY)
                else:
                    nc.vector.tensor_scalar(h8[:, 0, 0:3, :], tb[:], 0.0,
                                            None, op0=ALU.add)
                nc.vector.scalar_tensor_tensor(
                    h8[:, 1, 0:3, :], tb[:], 1.0, h8[:, 0, 0:3, :],
                    op0=ALU.mult, op1=ALU.subtract)
                nc.scalar.activation(
                    h8[0:16, 0, 3, :], psums[3][:], RELU,
                    bias=bias[:16, bcol + 3:bcol + 4],
                    scale=scal[0:16, scol:scol + 1])
                return h8

            # ---- heads (unchanged from baseline) -----------------------
            M_tiles = {}
            prev_h1 = None

            def head_act(dst, psum, bcol, t, i):
                bap = bias[:HW1, bcol + t:bcol + t + 1]
                if i % 2 == 0:
                    nc.scalar.activation(dst, psum, RELU, bias=bap,
                                         scale=1.0)
                else:
                    nc.vector.tensor_scalar(
                        dst, psum, bap, 0.0, op0=ALU.add, op1=ALU.max)

            def alloc_m(ci):
                sa = segs_all[ci]
                sb = segs_all[ci - 1] if ci >= 1 else []
                tiles = []
                for k in range(max(len(sa), len(sb))):
                    w1_ = sa[k][1] - sa[k][0] if k < len(sa) else 0
                    w2_ = sb[k][1] - sb[k][0] if k < len(sb) else 0
                    mw = max(w1_, w2_)
                    mt = ap.tile([64, mw], BF16, tag=f"mm{k}", name="mt")
                    nc.gpsimd.memset(mt[:], 0.0)
                    tiles.append((mt, mw))
                M_tiles[ci + 1] = tiles

            def make_h1(ci, x2):
                def run():
                    mts = M_tiles[ci + 1]
                    for k, (s0, s1, t) in enumerate(segs_all[ci]):
                        w = s1 - s0
                        pt = ps.tile([HW1, w], F32, tag="ps", name="ph1")
                        for ki in range(H1_K):
                            nc.tensor.matmul(
                                pt[:], wh1[:, ki, HW1 * t:HW1 * (t + 1)],
                                x2[ki][:, s0:s1],
                                start=(ki == 0), stop=(ki == H1_K - 1))
                        head_act(mts[k][0][0:HW1, 0:w], pt[:], BC_H1B, t, k)
                return run

            def run_merge(ci):
                sa = segs_all[ci - 1]
                sb = (segs_all[ci - 2]
                      if ci >= 2 and ci != len(widths) else [])
                if not sa and not sb:
                    return
                m_in = M_tiles[ci]
                npair = min(len(sa), len(sb))
                ot = None
                if sb:
                    cw2 = widths[ci - 2]
                    ot = op.tile([NCLS, cw2], F32, tag="o")
                acts = []
                outs = []
                for k in range(max(len(sa), len(sb))):
                    mt, mw = m_in[k]
                    if k < npair:
                        t1, t2 = sa[k][2], sb[k][2]
                        pc = pair_cols[(t1, t2)]
                        pt = ps.tile([42, mw], F32, tag="ps", name="pmg")
                        nc.tensor.matmul(
                            pt[:], wh23[:, 42 * pc:42 * (pc + 1)],
                            mt[0:64, 0:mw], start=True, stop=True)
                        acts.append((pt, 0, k))
                        outs.append((pt, 32, k))
                    elif k < len(sa):
                        t1 = sa[k][2]
                        w = sa[k][1] - sa[k][0]
                        pt = ps.tile([HW1, w], F32, tag="ps", name="ph2")
                        nc.tensor.matmul(
                            pt[:], wh2[:, HW1 * t1:HW1 * (t1 + 1)],
                            mt[0:HW1, 0:w], start=True, stop=True)
                        acts.append((pt, 0, k))
                    else:
                        t2 = sb[k][2]
                        w = sb[k][1] - sb[k][0]
                        pt = ps.tile([NCLS, w], F32, tag="ps", name="ph3")
                        nc.tensor.matmul(
                            pt[:], wh3[:, NCLS * t2:NCLS * (t2 + 1)],
                            mt[0:64, 0:w], start=True, stop=True)
                        outs.append((pt, 0, k))
                for pt, r0, k in acts:
                    s0, s1, t1 = sa[k]
                    w = s1 - s0
                    head_act(M_tiles[ci + 1][k][0][32:60, 0:w],
                             pt[r0:r0 + HW1, 0:w], BC_H2B, t1, k)
                for pt, r0, k in outs:
                    s0, s1, t2 = sb[k]
                    w = s1 - s0
                    nc.vector.tensor_scalar(
                        ot[:, s0:s1], pt[r0:r0 + NCLS, 0:w],
                        bias[:NCLS, BC_H3B + t2:BC_H3B + t2 + 1], None,
                        op0=ALU.add)
                if sb:
                    nc.sync.dma_start(
                        out_d[:, starts[ci - 2]:
                              starts[ci - 2] + widths[ci - 2]], ot[:])

            # software pipeline across rounds: L1@r, L2@r-1, L3@r-2; all
            # head machinery (h1 / merge / early-h3) keeps its baseline
            # relative structure but indexed by bci = r - 2.
            nch = len(widths)
            h8s, l2o8s, pas, has = {}, {}, {}, {}
            for r in range(nch + 4):
                bci = r - 2
                if r < nch:
                    cw = widths[r]
                    xk = x0 if r == 0 else (x1 if r == 1 else
                                            load_x_chunk(starts[r], cw))
                    ps1 = mm_dr(xk, sa1, sb1, L1_M, cw, k_outer=(r == 0),
                                sc=sc1)
                    pas[r] = act_bf16(ps1[0:2], BC_L1, L1_M[0:2], "l1p", cw,
                                      eng="ad", pool=app)
                    h8s[r] = act_split(ps1[2:6], BC_L1 + 2, 0, "h", cw)
                if prev_h1 is not None:
                    prev_h1()
                    prev_h1 = None
                if 0 <= bci <= nch:
                    alloc_m(bci)
                if 0 <= r - 1 < nch:
                    c1_ = r - 1
                    ps2 = mm_dr(h8s.pop(c1_), sa2, sb2, L2_M, widths[c1_],
                                sc=sc2)
                    l2o8s[c1_] = act_split(ps2, BC_L2, 1, "l2", widths[c1_],
                                           hh_on_act=True)
                if bci >= 1:
                    run_merge(bci)
                if 0 <= r - 2 < nch:
                    c2_ = r - 2
                    ps3 = mm_dr(l2o8s.pop(c2_), sa3, sb3, L3_M, widths[c2_])
                    has[c2_] = act_bf16(ps3, BC_L3, L3_M, "l3o", widths[c2_],
                                        eng="da")
                    prev_h1 = make_h1(c2_, pas.pop(c2_) + has.pop(c2_))
                if bci == nch - 1:
                    sb_ = segs_all[bci - 1]
                    cw2 = widths[bci - 1]
                    ot = op.tile([NCLS, cw2], F32, tag="o")
                    pts = []
                    for k, (s0, s1, t2) in enumerate(sb_):
                        mt, mw = M_tiles[bci + 1][k]
                        pt = ps.tile([NCLS, s1 - s0], F32, tag="ps",
                                     name="ph3e")
                        nc.tensor.matmul(
                            pt[:], wh3[:, NCLS * t2:NCLS * (t2 + 1)],
                            mt[0:64, 0:s1 - s0], start=True, stop=True)
                        pts.append(pt)
                    for k, (s0, s1, t2) in enumerate(sb_):
                        nc.vector.tensor_scalar(
                            ot[:, s0:s1], pts[k][:],
                            bias[:NCLS, BC_H3B + t2:BC_H3B + t2 + 1], None,
                            op0=ALU.add)
                    nc.sync.dma_start(
                        out_d[:, starts[bci - 1]:starts[bci - 1] + cw2],
                        ot[:])

    nc.compile()
    return nc


def _plan(tt):
    tt = np.asarray(tt).astype(np.int64).reshape(B)
    counts = np.bincount(tt, minlength=T)
    g = -(-counts // NCORES)
    g[T - 1] += (-int(g.sum())) % 8
    rp = int(g.sum())
    order = np.argsort(tt, kind="stable")
    pos = 0
    chunks_idx = []
    for t in range(T):
        idx = order[pos:pos + counts[t]]
        pos += counts[t]
        need = NCORES * int(g[t])
        if need > len(idx):
            idx = np.concatenate([idx, np.repeat(idx[:1], need - len(idx))])
        chunks_idx.append(idx.reshape(NCORES, int(g[t])))
    perms = [np.concatenate([chunks_idx[t][c] for t in range(T)])
             for c in range(NCORES)]
    bounds = tuple(int(v) for v in np.concatenate([[0], np.cumsum(g)]))
    return rp, bounds, perms


def _pw2(target, am):
    am = float(am)
    if am < 1e-30:
        return np.float32(1.0)
    return np.float32(2.0 ** np.floor(np.log2(target / am)))


def _split8(a):
    """hi/lo e4m3 split (shared scale domain); returns (hi_e4, lo_e4)."""
    ah = np.asarray(a, E4NP)
    al = np.asarray(a - ah.astype(np.float32), E4NP)
    return ah, al


def _planes_AB(Ws, nfull, ktail):
    """Build SA [(2*nA*128), m] and SB [(2*nB*128), m] host arrays for a
    scaled weight matrix Ws [K, m] with K = nfull*128 + ktail.  Even nfull
    (L1): the packed k-tail is its own A plane (paired with zeros); odd
    nfull (L2/L3): the tail rides in B's last pair slot against the packed
    moving tail plane."""
    K, m = Ws.shape
    Wh, Wl = _split8(Ws)
    tail_in_a = (nfull % 2 == 0)
    nA = nfull + (1 if tail_in_a else 0)
    nB = (nfull + (0 if tail_in_a else 1)) // 2
    SA = np.zeros((2, nA, 128, m), E4NP)
    SB = np.zeros((2, nB, 128, m), E4NP)
    for j in range(nfull):
        SA[0, j] = Wh[128 * j:128 * (j + 1)]
        SA[1, j] = Wh[128 * j:128 * (j + 1)]
    r0 = 128 * nfull

    if tail_in_a:
        # L1: full residual tail (moving planes are host-built for free):
        # 3-fragment pack at partition offsets 0/32/64.
        SA[0, nfull, 0:ktail] = Wh[r0:r0 + ktail]
        SA[0, nfull, 32:32 + ktail] = Wh[r0:r0 + ktail]
        SA[0, nfull, 64:64 + ktail] = Wl[r0:r0 + ktail]
    for q in range(nB):
        for i in range(2):
            j = 2 * q + i
            if j < nfull:
                SB[i, q] = Wl[128 * j:128 * (j + 1)]
            elif j == nfull and not tail_in_a:
                # L2/L3: plain-fp8 tail (no residual) -- the device-side
                # tail activation is a single direct ACT write.
                SB[i, q, 0:ktail] = Wh[r0:r0 + ktail]
    # SC: plain-fp8 stationaries for the last (16-wide) m-tile -- Wh pairs
    # over the full k (no residual terms), restricted to the tail columns.
    # For even nfull (L1) the packed k-tail A-plane is reused on device.
    mt = 16
    nC = (nfull + (0 if tail_in_a else 1)) // 2
    SC = np.zeros((2, nC, 128, mt), E4NP)
    for q in range(nC):
        for i in range(2):
            j = 2 * q + i
            if j < nfull:
                SC[i, q] = Wh[128 * j:128 * (j + 1), m - mt:]
            elif j == nfull:
                SC[i, q, 0:ktail] = Wh[r0:r0 + ktail, m - mt:]
    return (SA.reshape(2 * nA * 128, m), SB.reshape(2 * nB * 128, m),
            SC.reshape(2 * nC * 128, mt))


def _prepare_inputs(rp, perms, pair_cols, x_s, task_id,
                    fc1_w, fc1_b, fc2_w, fc2_b, fc3_w, fc3_b,
                    priv_w, priv_b, h1_w, h1_b, h2_w, h2_b, h3_w, h3_b):
    f = np.float32
    task_id = int(task_id)
    relu = lambda v: np.maximum(v, 0)

    x2d = np.asarray(x_s, f).reshape(B, D)
    W1cat = np.concatenate([np.asarray(priv_w[task_id], f),
                            np.asarray(fc1_w, f)], axis=1)
    W2 = np.asarray(fc2_w, f)
    W3 = np.asarray(fc3_w, f)
    b1v = np.concatenate([np.asarray(priv_b[task_id], f),
                          np.asarray(fc1_b, f)])
    b2v, b3v = np.asarray(fc2_b, f), np.asarray(fc3_b, f)

    # scales: exact for inputs/weights, 4x-margin sampled for activations
    sx = _pw2(224, np.abs(x2d).max())
    sw1 = _pw2(224, np.abs(W1cat).max())
    sw2 = _pw2(224, np.abs(W2).max())
    sw3 = _pw2(224, np.abs(W3).max())
    xs_ = x2d[:512]
    hs_ = relu(xs_ @ fc1_w + fc1_b)
    sh = _pw2(56, np.abs(hs_).max())
    h2s = relu(hs_ @ W2 + b2v)
    s2 = _pw2(56, np.abs(h2s).max())

    SA1, SB1, SC1 = _planes_AB(W1cat * sw1, 6, 16)
    SA2, SB2, SC2 = _planes_AB(W2 * sw2, 3, 16)
    SA3, SB3, _ = _planes_AB(W3 * sw3, 3, 16)

    # x split (scaled): planes row-major (i, j, p)
    xh, xl = _split8(x2d * sx)

    # heads: p is stored at psum scale (sx*sw1), L3-out at (s2*sw3);
    # fold the descale into wh1 rows.
    wh1 = np.zeros((2 * LAT, T * HW1), BF16NP)
    wh2 = np.zeros((HW1, T * HW1), BF16NP)
    wh3 = np.zeros((64, T * NCLS), BF16NP)
    for t in range(T):
        w1t = np.asarray(h1_w[t], f).copy()
        w1t[0:LAT] /= (sx * sw1)
        w1t[LAT:] /= (s2 * sw3)
        wh1[:, HW1 * t:HW1 * (t + 1)] = w1t
        wh2[:, HW1 * t:HW1 * (t + 1)] = np.asarray(h2_w[t], f)
        wh3[32:60, NCLS * t:NCLS * (t + 1)] = np.asarray(h3_w[t], f)
    wh23 = np.zeros((64, 42 * max(1, len(pair_cols))), BF16NP)
    for (t1, t2), p in pair_cols.items():
        wh23[0:HW1, 42 * p:42 * p + HW1] = np.asarray(h2_w[t1], f)
        wh23[32:60, 42 * p + 32:42 * (p + 1)] = np.asarray(h3_w[t2], f)

    bias = np.zeros((128, NBC), f)

    def col_bias(v, msizes, col):
        r0 = 0
        for mp_ in msizes:
            bias[:mp_, col] = v[r0:r0 + mp_]
            r0 += mp_
            col += 1

    # p bias at psum scale; h bias at sh; L2 at s2; L3 at psum scale
    b1s = b1v.copy()
    b1s[0:LAT] *= (sx * sw1)
    b1s[LAT:] *= sh
    col_bias(b1s, L1_M, BC_L1)
    col_bias(b2v * s2, L2_M, BC_L2)
    col_bias(b3v * (s2 * sw3), L3_M, BC_L3)
    for t in range(T):
        bias[:HW1, BC_H1B + t] = np.asarray(h1_b[t], f)
        bias[:HW1, BC_H2B + t] = np.asarray(h2_b[t], f)
        bias[:NCLS, BC_H3B + t] = np.asarray(h3_b[t], f)

    scal = np.zeros((128, 2), f)
    scal[:, 0] = sh / (sx * sw1)
    scal[:, 1] = s2 / (sh * sw2)

    shared = {"sa1": SA1, "sb1": SB1, "sa2": SA2, "sb2": SB2,
              "sa3": SA3, "sb3": SB3, "sc1": SC1, "sc2": SC2,
              "wh1": wh1, "wh2": wh2, "wh3": wh3, "wh23": wh23,
              "bias": bias, "scal": scal}

    in_maps = []
    for c in range(NCORES):
        xT = np.zeros((XROWS, rp), E4NP)
        ph = xh[perms[c]].T      # [784, rp] e4m3
        pl = xl[perms[c]].T
        xT[0:768] = ph[0:768]
        xT[896:896 + 768] = pl[0:768]
        # tail pack at plane (0, 6): rows 768:784 -> 768+0:16/32:48/64:80
        base = 6 * 128
        xT[base + 0:base + 16] = ph[768:784]
        xT[base + 32:base + 48] = pl[768:784]
        xT[base + 64:base + 80] = ph[768:784]
        m = dict(shared)
        m["xT"] = xT
        in_maps.append(m)
    return in_maps


def run(inputs, trace=False, **kw):
    inputs = {k: v for k, v in inputs.items() if k != "x_p"}
    rp, bounds, perms = _plan(inputs["tt"])
    widths = tuple(_chunks(rp))
    key = (widths, bounds)
    if _cache.get("key") != key:
        _cache["nc"] = _build_program(widths, bounds)
        _cache["key"] = key
    nc = _cache["nc"]
    pair_cols = _mk_schedule(widths, bounds)[1]
    in_maps = _prepare_inputs(
        rp, perms, pair_cols,
        **{k: v for k, v in inputs.items() if k != "tt"})
    res = run_bass_kernel_spmd(nc, in_maps, list(range(NCORES)),
                               trace=trace, **kw)
    full = np.empty((B, NCLS), np.float32)
    for c in range(NCORES):
        full[perms[c]] = res.results[c]["out"].T
    return full, res


def kernel(**inputs):
    out, _ = run(inputs, trace=False)
    return out
